# revision 1
# baseline (speedup 1.0000x reference)
"""HAN layer (3-metapath GCN mean) Trainium2 Bass kernel, 8-core SPMD.

Strategy (dst-sharded pull):
  - nodes range-sharded across 8 cores (6250 each); every core computes the
    full x_m = (h * rsqrt(deg_out_m)) @ W_m table (bf16 matmul, fp32 result)
    into two DRAM tables per metapath of <32768 rows each (int16 index limit
    of dma_gather), each with 128 trailing zero rows for padding slots.
  - per (core, metapath, chunk): in-edges of owned nodes are laid out by the
    host into a gather slot schedule: nodes sorted by chunk-degree descending,
    blocks of 128 nodes, per-block fixed column count T[b] (elementwise max
    over the 8 cores so the program is identical across cores).  Batched
    4-queue dma_gather pulls [128, cols, 64] fp32 supertiles; VectorE
    tensor_reduce sums each block's columns; raw block sums are
    dma_scatter_add-ed (un-permuting) into a zeroed per-metapath DRAM
    aggregate.  A final canonical pass applies rsqrt(deg_in), bias, relu, 1/3
    and accumulates the three metapaths into the output.
  - host concatenates the 8 core outputs.
"""

import numpy as np
import ml_dtypes

import concourse.bass as bass
import concourse.tile as tile
from concourse import bacc, mybir
from concourse.bass_utils import run_bass_kernel_spmd

F_IN, F_OUT, NMP = 128, 64, 3
GROUP_MAX_COLS = 64  # max supertile columns per dma_gather call


def _wrap16(flat):
    """slot i -> (partition i%16, free i//16), replicated to 128 partitions."""
    a = flat.astype(np.int16).reshape(-1, 16).T.copy()  # [16, S/16]
    return np.tile(a, (8, 1))


class _NS:
    pass


def _make_plan(N, ncores):
    p = _NS()
    p.N, p.ncores = N, ncores
    p.npc = N // ncores
    p.NBP = (p.npc + 127) // 128
    p.npc_pad = p.NBP * 128
    p.NT = (N + 511) // 512 * 4
    p.N_pad = p.NT * 128
    p.tilesA = (p.NT + 1) // 2
    p.tilesB = p.NT - p.tilesA
    p.CHN = p.tilesA * 128
    p.rowsA = p.tilesA * 128 + 128
    p.rowsB = p.tilesB * 128 + 128
    p.zeroA = p.tilesA * 128
    p.zeroB = p.tilesB * 128
    assert p.rowsA < 32768 and p.rowsB < 32768
    return p


def _build_stream(plan, d_sel, li_sel, order, Ts, zero_base):
    TOT = int(Ts.sum())
    fill = (zero_base + (np.arange(TOT * 128) % 128)).astype(np.int16)
    if TOT == 0 or len(d_sel) == 0:
        return fill
    B = np.zeros(plan.NBP, np.int64)
    B[1:] = np.cumsum(Ts)[:-1]
    rank = np.empty(plan.npc, np.int64)
    rank[order] = np.arange(plan.npc)
    r_e = rank[d_sel]
    o = np.argsort(r_e, kind="stable")
    r_s = r_e[o]
    li = li_sel[o]
    starts = np.searchsorted(r_s, np.arange(plan.npc))
    k = np.arange(len(r_s)) - starts[r_s]
    blk = r_s // 128
    col = B[blk] + k
    assert (k < Ts[blk]).all()
    fill[col * 128 + (r_s % 128)] = li.astype(np.int16)
    return fill


def _groups(Ts):
    out, b, col, NB = [], 0, 0, len(Ts)
    while b < NB:
        if Ts[b] == 0:
            b += 1
            continue
        b_lo, col_lo, ncols = b, col, 0
        while b < NB and (ncols == 0 or ncols + Ts[b] <= GROUP_MAX_COLS):
            ncols += int(Ts[b])
            col += int(Ts[b])
            b += 1
        out.append((b_lo, b, col_lo, ncols))
    return out


def _prepare(plan, h, srcs, dsts, Ws, bs):
    N, npc = plan.N, plan.npc
    h_pad = np.zeros((plan.N_pad, F_IN), np.float32)
    h_pad[:N] = h
    hT = np.ascontiguousarray(h_pad.T).astype(ml_dtypes.bfloat16)

    rs_out, rs_in = [], []
    for m in range(NMP):
        do = np.clip(np.bincount(srcs[m], minlength=N), 1, None).astype(np.float64)
        di = np.clip(np.bincount(dsts[m], minlength=N), 1, None).astype(np.float64)
        rs_out.append((1.0 / np.sqrt(do)).astype(np.float32))
        rs_in.append((1.0 / np.sqrt(di)).astype(np.float32))

    rso = np.zeros((128, plan.NT * NMP), np.float32)
    for m in range(NMP):
        v = np.zeros(plan.N_pad, np.float32)
        v[:N] = rs_out[m]
        rso[:, m::NMP] = v.reshape(plan.NT, 128).T

    wall = np.concatenate(Ws, axis=1).astype(ml_dtypes.bfloat16)
    ball = np.concatenate(
        [np.tile(b[None, :], (128, 1)) for b in bs], axis=1
    ).astype(np.float32)

    info = {}
    T0s = [np.zeros(plan.NBP, np.int64) for _ in range(NMP)]
    T1s = [np.zeros(plan.NBP, np.int64) for _ in range(NMP)]
    pad = plan.npc_pad - npc
    z = np.zeros(pad, np.int64)
    for c in range(plan.ncores):
        lo = c * npc
        for m in range(NMP):
            sel = (dsts[m] >= lo) & (dsts[m] < lo + npc)
            s = srcs[m][sel]
            d = dsts[m][sel] - lo
            in0 = s < plan.CHN
            c0 = np.bincount(d[in0], minlength=npc)
            c1 = np.bincount(d[~in0], minlength=npc)
            o0 = np.argsort(-c0, kind="stable")
            o1 = np.argsort(-c1, kind="stable")
            info[(c, m)] = (s, d, in0, o0, o1)
            T0s[m] = np.maximum(
                T0s[m], np.concatenate([c0[o0], z]).reshape(plan.NBP, 128).max(1)
            )
            T1s[m] = np.maximum(
                T1s[m], np.concatenate([c1[o1], z]).reshape(plan.NBP, 128).max(1)
            )

    cfg = _NS()
    cfg.plan = plan
    cfg.T0s, cfg.T1s = T0s, T1s
    cfg.g0 = [_groups(T0s[m]) for m in range(NMP)]
    cfg.g1 = [_groups(T1s[m]) for m in range(NMP)]
    cfg.TOT0 = [int(T0s[m].sum()) for m in range(NMP)]
    cfg.TOT1 = [int(T1s[m].sum()) for m in range(NMP)]

    in_maps = []
    for c in range(plan.ncores):
        im = {"hT": hT, "rso": rso, "wall": wall, "ball": ball}
        lo = c * npc
        for m in range(NMP):
            s, d, in0, o0, o1 = info[(c, m)]
            st0 = _build_stream(plan, d[in0], s[in0], o0, T0s[m], plan.zeroA)
            st1 = _build_stream(
                plan, d[~in0], s[~in0] - plan.CHN, o1, T1s[m], plan.zeroB
            )
            im[f"g0_{m}"] = (
                _wrap16(st0) if cfg.TOT0[m] else np.zeros((128, 8), np.int16)
            )
            im[f"g1_{m}"] = (
                _wrap16(st1) if cfg.TOT1[m] else np.zeros((128, 8), np.int16)
            )
            for ch, o in ((0, o0), (1, o1)):
                si = np.full(plan.npc_pad, -1, np.int64)
                si[:npc] = o
                im[f"si{ch}_{m}"] = _wrap16(si)
            full = np.zeros(plan.npc_pad, np.float32)
            full[:npc] = rs_in[m][lo : lo + npc]
            im[f"ri_{m}"] = full.reshape(plan.NBP, 128).T.copy()
        in_maps.append(im)
    return cfg, in_maps


def _build_kernel(cfg):
    plan = cfg.plan
    nc = bacc.Bacc(
        "TRN2",
        target_bir_lowering=False,
        debug=False,
        num_devices=plan.ncores,
        num_swdge_queues=4,
    )
    dt = mybir.dt
    hT_d = nc.dram_tensor("hT", (128, plan.N_pad), dt.bfloat16, kind="ExternalInput").ap()
    rso_d = nc.dram_tensor("rso", (128, plan.NT * NMP), dt.float32, kind="ExternalInput").ap()
    wall_d = nc.dram_tensor("wall", (128, F_OUT * NMP), dt.bfloat16, kind="ExternalInput").ap()
    ball_d = nc.dram_tensor("ball", (128, F_OUT * NMP), dt.float32, kind="ExternalInput").ap()
    g_d, si_d, ri_d, xA, xB, agg = {}, {}, {}, {}, {}, {}
    for m in range(NMP):
        w0 = max(cfg.TOT0[m] * 8, 8)
        w1 = max(cfg.TOT1[m] * 8, 8)
        g_d[(0, m)] = nc.dram_tensor(f"g0_{m}", (128, w0), dt.int16, kind="ExternalInput").ap()
        g_d[(1, m)] = nc.dram_tensor(f"g1_{m}", (128, w1), dt.int16, kind="ExternalInput").ap()
        si_d[(0, m)] = nc.dram_tensor(f"si0_{m}", (128, plan.npc_pad // 16), dt.int16, kind="ExternalInput").ap()
        si_d[(1, m)] = nc.dram_tensor(f"si1_{m}", (128, plan.npc_pad // 16), dt.int16, kind="ExternalInput").ap()
        ri_d[m] = nc.dram_tensor(f"ri_{m}", (128, plan.NBP), dt.float32, kind="ExternalInput").ap()
        xA[m] = nc.dram_tensor(f"xA_{m}", (plan.rowsA, F_OUT), dt.float32, kind="Internal").ap()
        xB[m] = nc.dram_tensor(f"xB_{m}", (plan.rowsB, F_OUT), dt.float32, kind="Internal").ap()
        agg[m] = nc.dram_tensor(f"agg_{m}", (plan.npc_pad, F_OUT), dt.float32, kind="Internal").ap()
    out_d = nc.dram_tensor("out", (plan.npc_pad, F_OUT), dt.float32, kind="ExternalOutput").ap()

    with tile.TileContext(nc) as tc:
        with tc.tile_pool(name="const", bufs=1) as constp, \
             tc.tile_pool(name="ht", bufs=2) as htp, \
             tc.tile_pool(name="ps", bufs=4, space="PSUM") as psp, \
             tc.tile_pool(name="xs", bufs=3) as xsp, \
             tc.tile_pool(name="gidx", bufs=4) as gip, \
             tc.tile_pool(name="gath", bufs=4) as gp, \
             tc.tile_pool(name="accs", bufs=2) as accp, \
             tc.tile_pool(name="fin", bufs=2) as finp, \
             tc.tile_pool(name="mean", bufs=1) as meanp:

            wall_t = constp.tile([128, F_OUT * NMP], dt.bfloat16)
            nc.sync.dma_start(wall_t[:], wall_d[:])
            ball_t = constp.tile([128, F_OUT * NMP], dt.float32)
            nc.sync.dma_start(ball_t[:], ball_d[:])
            rso_t = constp.tile([128, plan.NT * NMP], dt.float32)
            nc.sync.dma_start(rso_t[:], rso_d[:])
            ri_t, si_t = {}, {}
            for m in range(NMP):
                ri_t[m] = constp.tile([128, plan.NBP], dt.float32, name=f"ri_t{m}")
                nc.sync.dma_start(ri_t[m][:], ri_d[m][:])
                for ch in (0, 1):
                    si_t[(ch, m)] = constp.tile(
                        [128, plan.npc_pad // 16], dt.int16, name=f"si_t{ch}_{m}"
                    )
                    nc.sync.dma_start(si_t[(ch, m)][:], si_d[(ch, m)][:])

            zt = constp.tile([128, F_OUT], dt.float32)
            nc.vector.memset(zt[:], 0.0)
            for m in range(NMP):
                nc.sync.dma_start(xA[m][plan.zeroA : plan.zeroA + 128, :], zt[:])
                nc.sync.dma_start(xB[m][plan.zeroB : plan.zeroB + 128, :], zt[:])
            zo = constp.tile([128, plan.npc_pad * F_OUT // 128], dt.float32)
            nc.vector.memset(zo[:], 0.0)
            for m in range(NMP):
                nc.sync.dma_start(
                    agg[m][:].rearrange("n f -> (n f)").rearrange(
                        "(p x) -> p x", p=128
                    ),
                    zo[:],
                )

            # ---- phase 1: x_m tables ----
            SLAB = 4
            for slab in range(plan.N_pad // 512):
                hts = htp.tile([128, 512], dt.bfloat16)
                nc.sync.dma_start(hts[:], hT_d[:, slab * 512 : (slab + 1) * 512])
                stg = xsp.tile([128, NMP, SLAB, F_OUT], dt.float32)
                for j in range(SLAB):
                    t = slab * SLAB + j
                    ps = psp.tile([128, F_OUT * NMP], dt.float32, space="PSUM")
                    nc.tensor.matmul(
                        ps[:],
                        lhsT=hts[:, j * 128 : (j + 1) * 128],
                        rhs=wall_t[:],
                        start=True,
                        stop=True,
                    )
                    for m in range(NMP):
                        nc.vector.tensor_scalar(
                            stg[:, m, j, :],
                            ps[:, m * F_OUT : (m + 1) * F_OUT],
                            rso_t[:, t * NMP + m : t * NMP + m + 1],
                            None,
                            mybir.AluOpType.mult,
                        )
                t0 = slab * SLAB
                for m in range(NMP):
                    if t0 < plan.tilesA:
                        tab, base = xA[m], t0 * 128
                    else:
                        tab, base = xB[m], (t0 - plan.tilesA) * 128
                    nc.sync.dma_start(
                        tab[base : base + SLAB * 128, :].rearrange(
                            "(s p) f -> p s f", p=128
                        ),
                        stg[:, m, :, :],
                    )

            # ---- phase 2: gather + segment reduce + scatter into agg ----
            qrr = 0
            for m in range(NMP):
                for ch, groups, Ts, tabs in (
                    (0, cfg.g0[m], cfg.T0s[m], xA),
                    (1, cfg.g1[m], cfg.T1s[m], xB),
                ):
                    acc = accp.tile(
                        [128, plan.NBP, F_OUT], dt.float32, name="acc"
                    )
                    nc.vector.memset(acc[:], 0.0)
                    Bcols = np.zeros(plan.NBP, np.int64)
                    Bcols[1:] = np.cumsum(Ts)[:-1]
                    for (b_lo, b_hi, col_lo, ncols) in groups:
                        it = gip.tile([128, ncols * 8], dt.int16, name="it")
                        nc.sync.dma_start(
                            it[:],
                            g_d[(ch, m)][:, col_lo * 8 : (col_lo + ncols) * 8],
                        )
                        gt = gp.tile([128, ncols, F_OUT], dt.float32, name="gt")
                        nc.gpsimd.dma_gather(
                            out_ap=gt[:],
                            in_ap=tabs[m][:],
                            idxs_ap=it[:],
                            num_idxs=ncols * 128,
                            num_idxs_reg=ncols * 128,
                            elem_size=F_OUT,
                            single_packet=False,
                            queue_num=qrr,
                        )
                        qrr = (qrr + 1) % 4
                        for b in range(b_lo, b_hi):
                            cl = int(Bcols[b] - col_lo)
                            w = int(Ts[b])
                            view = gt[:, cl : cl + w, :].rearrange("p t f -> p f t")
                            nc.vector.tensor_reduce(
                                acc[:, b, :], view, mybir.AxisListType.X,
                                mybir.AluOpType.add,
                            )
                    nc.gpsimd.dma_scatter_add(
                        out_ap=agg[m][:],
                        in_ap=acc[:],
                        idxs_ap=si_t[(ch, m)][:],
                        num_idxs=plan.npc_pad,
                        num_idxs_reg=plan.npc,
                        elem_size=F_OUT,
                        single_packet=False,
                        queue_num=qrr,
                    )
                    qrr = (qrr + 1) % 4

            # ---- phase 3: canonical finalize + mean ----
            mean_t = meanp.tile([128, plan.NBP, F_OUT], dt.float32)
            nc.vector.memset(mean_t[:], 0.0)
            for m in range(NMP):
                fin = finp.tile([128, plan.NBP, F_OUT], dt.float32, name="fin")
                nc.sync.dma_start(
                    fin[:],
                    agg[m][:].rearrange("(t p) f -> p t f", p=128),
                )
                for b in range(plan.NBP):
                    nc.vector.tensor_scalar(
                        fin[:, b, :], fin[:, b, :], ri_t[m][:, b : b + 1], None,
                        mybir.AluOpType.mult,
                    )
                    nc.vector.tensor_tensor(
                        fin[:, b, :], fin[:, b, :],
                        ball_t[:, m * F_OUT : (m + 1) * F_OUT],
                        mybir.AluOpType.add,
                    )
                    nc.scalar.activation(
                        fin[:, b, :], fin[:, b, :],
                        mybir.ActivationFunctionType.Relu,
                        scale=1.0 / 3.0,
                    )
                nc.vector.tensor_tensor(
                    mean_t[:], mean_t[:], fin[:], mybir.AluOpType.add
                )
            nc.sync.dma_start(
                out_d[:].rearrange("(t p) f -> p t f", p=128), mean_t[:]
            )
    nc.compile()
    return nc


_CACHE = {}


def _get_compiled(plan, h, srcs, dsts, Ws, bs):
    cfg, in_maps = _prepare(plan, h, srcs, dsts, Ws, bs)
    key = (
        plan.N,
        plan.ncores,
        tuple(tuple(t) for t in cfg.T0s),
        tuple(tuple(t) for t in cfg.T1s),
    )
    if key not in _CACHE:
        _CACHE[key] = _build_kernel(cfg)
    return _CACHE[key], cfg, in_maps


def run(h, srcs, dsts, Ws, bs, N=None, ncores=8, trace=False):
    N = h.shape[0] if N is None else N
    plan = _make_plan(N, ncores)
    nc, cfg, in_maps = _get_compiled(plan, h, srcs, dsts, Ws, bs)
    res = run_bass_kernel_spmd(
        nc, in_maps, core_ids=list(range(ncores)), trace=trace
    )
    out = np.concatenate(
        [res.results[c]["out"][: plan.npc] for c in range(ncores)], axis=0
    )
    return out[:N], res


def kernel(h, src0, dst0, src1, dst1, src2, dst2, W0, b0, W1, b1, W2, b2):
    h = np.asarray(h, np.float32)
    srcs = [np.asarray(s, np.int64) for s in (src0, src1, src2)]
    dsts = [np.asarray(d, np.int64) for d in (dst0, dst1, dst2)]
    Ws = [np.asarray(w, np.float32) for w in (W0, W1, W2)]
    bs = [np.asarray(b, np.float32) for b in (b0, b1, b2)]
    out, _ = run(h, srcs, dsts, Ws, bs)
    return out.astype(np.float32)



# revision 2
# speedup vs baseline: 1.0556x; 1.0556x over previous
"""HAN layer (3-metapath GCN mean) Trainium2 Bass kernel, 8-core SPMD.

Strategy (dst-sharded pull):
  - nodes range-sharded across 8 cores (6250 each); every core computes the
    full x_m = (h * rsqrt(deg_out_m)) @ W_m table (bf16 matmul, fp32 result)
    into two DRAM tables per metapath of <32768 rows each (int16 index limit
    of dma_gather), each with 128 trailing zero rows for padding slots.
  - per (core, metapath, chunk): in-edges of owned nodes are laid out by the
    host into a gather slot schedule: nodes sorted by chunk-degree descending,
    blocks of 128 nodes, per-block fixed column count T[b] (elementwise max
    over the 8 cores so the program is identical across cores).  Batched
    4-queue dma_gather pulls [128, cols, 64] fp32 supertiles; VectorE
    tensor_reduce sums each block's columns; raw block sums are
    dma_scatter_add-ed (un-permuting) into a zeroed per-metapath DRAM
    aggregate.  A final canonical pass applies rsqrt(deg_in), bias, relu, 1/3
    and accumulates the three metapaths into the output.
  - host concatenates the 8 core outputs.
"""

import numpy as np
import ml_dtypes

import concourse.bass as bass
import concourse.tile as tile
from concourse import bacc, mybir
from concourse.bass_utils import run_bass_kernel_spmd

F_IN, F_OUT, NMP = 128, 64, 3
GROUP_MAX_COLS = 64  # max supertile columns per dma_gather call


def _wrap16(flat):
    """slot i -> (partition i%16, free i//16), replicated to 128 partitions."""
    a = flat.astype(np.int16).reshape(-1, 16).T.copy()  # [16, S/16]
    return np.tile(a, (8, 1))


class _NS:
    pass


def _make_plan(N, ncores):
    p = _NS()
    p.N, p.ncores = N, ncores
    p.npc = N // ncores
    p.NBP = (p.npc + 127) // 128
    p.npc_pad = p.NBP * 128
    p.NT = (N + 511) // 512 * 4
    p.N_pad = p.NT * 128
    p.tilesA = (p.NT + 1) // 2
    p.tilesB = p.NT - p.tilesA
    p.CHN = p.tilesA * 128
    p.rowsA = p.tilesA * 128 + 128
    p.rowsB = p.tilesB * 128 + 128
    p.zeroA = p.tilesA * 128
    p.zeroB = p.tilesB * 128
    assert p.rowsA < 32768 and p.rowsB < 32768
    return p


def _build_stream(plan, d_sel, li_sel, order, Ts, zero_base):
    TOT = int(Ts.sum())
    fill = (zero_base + (np.arange(TOT * 128) % 128)).astype(np.int16)
    if TOT == 0 or len(d_sel) == 0:
        return fill
    B = np.zeros(plan.NBP, np.int64)
    B[1:] = np.cumsum(Ts)[:-1]
    rank = np.empty(plan.npc, np.int64)
    rank[order] = np.arange(plan.npc)
    r_e = rank[d_sel]
    o = np.argsort(r_e, kind="stable")
    r_s = r_e[o]
    li = li_sel[o]
    starts = np.searchsorted(r_s, np.arange(plan.npc))
    k = np.arange(len(r_s)) - starts[r_s]
    blk = r_s // 128
    col = B[blk] + k
    assert (k < Ts[blk]).all()
    fill[col * 128 + (r_s % 128)] = li.astype(np.int16)
    return fill


def _groups(Ts):
    out, b, col, NB = [], 0, 0, len(Ts)
    while b < NB:
        if Ts[b] == 0:
            b += 1
            continue
        b_lo, col_lo, ncols = b, col, 0
        while b < NB and (ncols == 0 or ncols + Ts[b] <= GROUP_MAX_COLS):
            ncols += int(Ts[b])
            col += int(Ts[b])
            b += 1
        out.append((b_lo, b, col_lo, ncols))
    return out


def _prepare(plan, h, srcs, dsts, Ws, bs):
    N, npc = plan.N, plan.npc
    h_pad = np.zeros((plan.N_pad, F_IN), np.float32)
    h_pad[:N] = h
    hT = np.ascontiguousarray(h_pad.T).astype(ml_dtypes.bfloat16)

    rs_out, rs_in = [], []
    for m in range(NMP):
        do = np.clip(np.bincount(srcs[m], minlength=N), 1, None).astype(np.float64)
        di = np.clip(np.bincount(dsts[m], minlength=N), 1, None).astype(np.float64)
        rs_out.append((1.0 / np.sqrt(do)).astype(np.float32))
        rs_in.append((1.0 / np.sqrt(di)).astype(np.float32))

    rso = np.zeros((128, plan.NT * NMP), np.float32)
    for m in range(NMP):
        v = np.zeros(plan.N_pad, np.float32)
        v[:N] = rs_out[m]
        rso[:, m::NMP] = v.reshape(plan.NT, 128).T

    wall = np.concatenate(Ws, axis=1).astype(ml_dtypes.bfloat16)
    ball = np.concatenate(
        [np.tile(b[None, :], (128, 1)) for b in bs], axis=1
    ).astype(np.float32)

    info = {}
    T0s = [np.zeros(plan.NBP, np.int64) for _ in range(NMP)]
    T1s = [np.zeros(plan.NBP, np.int64) for _ in range(NMP)]
    pad = plan.npc_pad - npc
    z = np.zeros(pad, np.int64)
    for c in range(plan.ncores):
        lo = c * npc
        for m in range(NMP):
            sel = (dsts[m] >= lo) & (dsts[m] < lo + npc)
            s = srcs[m][sel]
            d = dsts[m][sel] - lo
            in0 = s < plan.CHN
            c0 = np.bincount(d[in0], minlength=npc)
            c1 = np.bincount(d[~in0], minlength=npc)
            o0 = np.argsort(-c0, kind="stable")
            o1 = np.argsort(-c1, kind="stable")
            info[(c, m)] = (s, d, in0, o0, o1)
            T0s[m] = np.maximum(
                T0s[m], np.concatenate([c0[o0], z]).reshape(plan.NBP, 128).max(1)
            )
            T1s[m] = np.maximum(
                T1s[m], np.concatenate([c1[o1], z]).reshape(plan.NBP, 128).max(1)
            )

    cfg = _NS()
    cfg.plan = plan
    cfg.T0s, cfg.T1s = T0s, T1s
    cfg.g0 = [_groups(T0s[m]) for m in range(NMP)]
    cfg.g1 = [_groups(T1s[m]) for m in range(NMP)]
    cfg.TOT0 = [int(T0s[m].sum()) for m in range(NMP)]
    cfg.TOT1 = [int(T1s[m].sum()) for m in range(NMP)]

    in_maps = []
    for c in range(plan.ncores):
        im = {"hT": hT, "rso": rso, "wall": wall, "ball": ball}
        lo = c * npc
        for m in range(NMP):
            s, d, in0, o0, o1 = info[(c, m)]
            st0 = _build_stream(plan, d[in0], s[in0], o0, T0s[m], plan.zeroA)
            st1 = _build_stream(
                plan, d[~in0], s[~in0] - plan.CHN, o1, T1s[m], plan.zeroB
            )
            im[f"g0_{m}"] = (
                _wrap16(st0) if cfg.TOT0[m] else np.zeros((128, 8), np.int16)
            )
            im[f"g1_{m}"] = (
                _wrap16(st1) if cfg.TOT1[m] else np.zeros((128, 8), np.int16)
            )
            for ch, o in ((0, o0), (1, o1)):
                si = np.full(plan.npc_pad, -1, np.int64)
                si[:npc] = o
                im[f"si{ch}_{m}"] = _wrap16(si)
            full = np.zeros(plan.npc_pad, np.float32)
            full[:npc] = rs_in[m][lo : lo + npc]
            im[f"ri_{m}"] = full.reshape(plan.NBP, 128).T.copy()
        in_maps.append(im)
    return cfg, in_maps


def _build_kernel(cfg):
    plan = cfg.plan
    nc = bacc.Bacc(
        "TRN2",
        target_bir_lowering=False,
        debug=False,
        num_devices=plan.ncores,
        num_swdge_queues=4,
    )
    dt = mybir.dt
    hT_d = nc.dram_tensor("hT", (128, plan.N_pad), dt.bfloat16, kind="ExternalInput").ap()
    rso_d = nc.dram_tensor("rso", (128, plan.NT * NMP), dt.float32, kind="ExternalInput").ap()
    wall_d = nc.dram_tensor("wall", (128, F_OUT * NMP), dt.bfloat16, kind="ExternalInput").ap()
    ball_d = nc.dram_tensor("ball", (128, F_OUT * NMP), dt.float32, kind="ExternalInput").ap()
    g_d, si_d, ri_d, xA, xB, agg = {}, {}, {}, {}, {}, {}
    for m in range(NMP):
        w0 = max(cfg.TOT0[m] * 8, 8)
        w1 = max(cfg.TOT1[m] * 8, 8)
        g_d[(0, m)] = nc.dram_tensor(f"g0_{m}", (128, w0), dt.int16, kind="ExternalInput").ap()
        g_d[(1, m)] = nc.dram_tensor(f"g1_{m}", (128, w1), dt.int16, kind="ExternalInput").ap()
        si_d[(0, m)] = nc.dram_tensor(f"si0_{m}", (128, plan.npc_pad // 16), dt.int16, kind="ExternalInput").ap()
        si_d[(1, m)] = nc.dram_tensor(f"si1_{m}", (128, plan.npc_pad // 16), dt.int16, kind="ExternalInput").ap()
        ri_d[m] = nc.dram_tensor(f"ri_{m}", (128, plan.NBP), dt.float32, kind="ExternalInput").ap()
        xA[m] = nc.dram_tensor(f"xA_{m}", (plan.rowsA, F_OUT), dt.float32, kind="Internal").ap()
        xB[m] = nc.dram_tensor(f"xB_{m}", (plan.rowsB, F_OUT), dt.float32, kind="Internal").ap()
        agg[m] = nc.dram_tensor(f"agg_{m}", (plan.npc_pad, F_OUT), dt.float32, kind="Internal").ap()
    out_d = nc.dram_tensor("out", (plan.npc_pad, F_OUT), dt.float32, kind="ExternalOutput").ap()

    with tile.TileContext(nc) as tc:
        with tc.tile_pool(name="const", bufs=1) as constp, \
             tc.tile_pool(name="ht", bufs=2) as htp, \
             tc.tile_pool(name="ps", bufs=4, space="PSUM") as psp, \
             tc.tile_pool(name="xs", bufs=3) as xsp, \
             tc.tile_pool(name="gidx", bufs=6) as gip, \
             tc.tile_pool(name="gath", bufs=6) as gp, \
             tc.tile_pool(name="accs", bufs=2) as accp, \
             tc.tile_pool(name="fin", bufs=2) as finp, \
             tc.tile_pool(name="mean", bufs=1) as meanp:

            wall_t = constp.tile([128, F_OUT * NMP], dt.bfloat16)
            nc.sync.dma_start(wall_t[:], wall_d[:])
            ball_t = constp.tile([128, F_OUT * NMP], dt.float32)
            nc.sync.dma_start(ball_t[:], ball_d[:])
            rso_t = constp.tile([128, plan.NT * NMP], dt.float32)
            nc.sync.dma_start(rso_t[:], rso_d[:])
            ri_t, si_t = {}, {}
            for m in range(NMP):
                ri_t[m] = constp.tile([128, plan.NBP], dt.float32, name=f"ri_t{m}")
                nc.sync.dma_start(ri_t[m][:], ri_d[m][:])
                for ch in (0, 1):
                    si_t[(ch, m)] = constp.tile(
                        [128, plan.npc_pad // 16], dt.int16, name=f"si_t{ch}_{m}"
                    )
                    nc.sync.dma_start(si_t[(ch, m)][:], si_d[(ch, m)][:])

            zt = constp.tile([128, F_OUT], dt.float32)
            nc.vector.memset(zt[:], 0.0)
            for m in range(NMP):
                nc.sync.dma_start(xA[m][plan.zeroA : plan.zeroA + 128, :], zt[:])
                nc.sync.dma_start(xB[m][plan.zeroB : plan.zeroB + 128, :], zt[:])
            zo = constp.tile([128, plan.npc_pad * F_OUT // 128], dt.float32)
            nc.vector.memset(zo[:], 0.0)
            for m in range(NMP):
                nc.sync.dma_start(
                    agg[m][:].rearrange("n f -> (n f)").rearrange(
                        "(p x) -> p x", p=128
                    ),
                    zo[:],
                )

            # ---- phase 1: x_m tables ----
            SLAB = 4
            for slab in range(plan.N_pad // 512):
                hts = htp.tile([128, 512], dt.bfloat16)
                nc.sync.dma_start(hts[:], hT_d[:, slab * 512 : (slab + 1) * 512])
                stg = xsp.tile([128, NMP, SLAB, F_OUT], dt.float32)
                for j in range(SLAB):
                    t = slab * SLAB + j
                    ps = psp.tile([128, F_OUT * NMP], dt.float32, space="PSUM")
                    nc.tensor.matmul(
                        ps[:],
                        lhsT=hts[:, j * 128 : (j + 1) * 128],
                        rhs=wall_t[:],
                        start=True,
                        stop=True,
                    )
                    for m in range(NMP):
                        nc.vector.tensor_scalar(
                            stg[:, m, j, :],
                            ps[:, m * F_OUT : (m + 1) * F_OUT],
                            rso_t[:, t * NMP + m : t * NMP + m + 1],
                            None,
                            mybir.AluOpType.mult,
                        )
                t0 = slab * SLAB
                for m in range(NMP):
                    if t0 < plan.tilesA:
                        tab, base = xA[m], t0 * 128
                    else:
                        tab, base = xB[m], (t0 - plan.tilesA) * 128
                    nc.sync.dma_start(
                        tab[base : base + SLAB * 128, :].rearrange(
                            "(s p) f -> p s f", p=128
                        ),
                        stg[:, m, :, :],
                    )

            # ---- phase 2: gather + segment reduce + scatter into agg ----
            qrr = 0
            for m in range(NMP):
                for ch, groups, Ts, tabs in (
                    (0, cfg.g0[m], cfg.T0s[m], xA),
                    (1, cfg.g1[m], cfg.T1s[m], xB),
                ):
                    acc = accp.tile(
                        [128, plan.NBP, F_OUT], dt.float32, name="acc"
                    )
                    nc.vector.memset(acc[:], 0.0)
                    Bcols = np.zeros(plan.NBP, np.int64)
                    Bcols[1:] = np.cumsum(Ts)[:-1]
                    for (b_lo, b_hi, col_lo, ncols) in groups:
                        it = gip.tile([128, ncols * 8], dt.int16, name="it")
                        nc.sync.dma_start(
                            it[:],
                            g_d[(ch, m)][:, col_lo * 8 : (col_lo + ncols) * 8],
                        )
                        gt = gp.tile([128, ncols, F_OUT], dt.float32, name="gt")
                        nc.gpsimd.dma_gather(
                            out_ap=gt[:],
                            in_ap=tabs[m][:],
                            idxs_ap=it[:],
                            num_idxs=ncols * 128,
                            num_idxs_reg=ncols * 128,
                            elem_size=F_OUT,
                            single_packet=False,
                            queue_num=qrr,
                        )
                        qrr = (qrr + 1) % 4
                        for b in range(b_lo, b_hi):
                            cl = int(Bcols[b] - col_lo)
                            w = int(Ts[b])
                            view = gt[:, cl : cl + w, :].rearrange("p t f -> p f t")
                            nc.vector.tensor_reduce(
                                acc[:, b, :], view, mybir.AxisListType.X,
                                mybir.AluOpType.add,
                            )
                    nc.gpsimd.dma_scatter_add(
                        out_ap=agg[m][:],
                        in_ap=acc[:],
                        idxs_ap=si_t[(ch, m)][:],
                        num_idxs=plan.npc_pad,
                        num_idxs_reg=plan.npc,
                        elem_size=F_OUT,
                        single_packet=False,
                        queue_num=qrr,
                    )
                    qrr = (qrr + 1) % 4

            # ---- phase 3: canonical finalize + mean ----
            mean_t = meanp.tile([128, plan.NBP, F_OUT], dt.float32)
            nc.vector.memset(mean_t[:], 0.0)
            for m in range(NMP):
                fin = finp.tile([128, plan.NBP, F_OUT], dt.float32, name="fin")
                nc.sync.dma_start(
                    fin[:],
                    agg[m][:].rearrange("(t p) f -> p t f", p=128),
                )
                for b in range(plan.NBP):
                    nc.vector.tensor_scalar(
                        fin[:, b, :], fin[:, b, :], ri_t[m][:, b : b + 1], None,
                        mybir.AluOpType.mult,
                    )
                    nc.vector.tensor_tensor(
                        fin[:, b, :], fin[:, b, :],
                        ball_t[:, m * F_OUT : (m + 1) * F_OUT],
                        mybir.AluOpType.add,
                    )
                    nc.scalar.activation(
                        fin[:, b, :], fin[:, b, :],
                        mybir.ActivationFunctionType.Relu,
                        scale=1.0 / 3.0,
                    )
                nc.vector.tensor_tensor(
                    mean_t[:], mean_t[:], fin[:], mybir.AluOpType.add
                )
            nc.sync.dma_start(
                out_d[:].rearrange("(t p) f -> p t f", p=128), mean_t[:]
            )
    nc.compile()
    return nc


_CACHE = {}


def _get_compiled(plan, h, srcs, dsts, Ws, bs):
    cfg, in_maps = _prepare(plan, h, srcs, dsts, Ws, bs)
    key = (
        plan.N,
        plan.ncores,
        tuple(tuple(t) for t in cfg.T0s),
        tuple(tuple(t) for t in cfg.T1s),
    )
    if key not in _CACHE:
        _CACHE[key] = _build_kernel(cfg)
    return _CACHE[key], cfg, in_maps


def run(h, srcs, dsts, Ws, bs, N=None, ncores=8, trace=False):
    N = h.shape[0] if N is None else N
    plan = _make_plan(N, ncores)
    nc, cfg, in_maps = _get_compiled(plan, h, srcs, dsts, Ws, bs)
    res = run_bass_kernel_spmd(
        nc, in_maps, core_ids=list(range(ncores)), trace=trace
    )
    out = np.concatenate(
        [res.results[c]["out"][: plan.npc] for c in range(ncores)], axis=0
    )
    return out[:N], res


def kernel(h, src0, dst0, src1, dst1, src2, dst2, W0, b0, W1, b1, W2, b2):
    h = np.asarray(h, np.float32)
    srcs = [np.asarray(s, np.int64) for s in (src0, src1, src2)]
    dsts = [np.asarray(d, np.int64) for d in (dst0, dst1, dst2)]
    Ws = [np.asarray(w, np.float32) for w in (W0, W1, W2)]
    bs = [np.asarray(b, np.float32) for b in (b0, b1, b2)]
    out, _ = run(h, srcs, dsts, Ws, bs)
    return out.astype(np.float32)

